# revision 18
# baseline (speedup 1.0000x reference)
"""GCN message-passing layer (copy_src -> segment_sum -> dual degree norm)
on 8 Trainium2 NeuronCores.

Strategy (dst-sharded message passing, v3):
  Host side (sharding/metadata only):
    - node_f = concat(u_f, v_f) * out_norm[src-side], cast to bf16.
      in-degree norm is applied to the FINAL output on the host (a
      per-dst-row scale commutes with the edge aggregation), so on-device
      one-hot matrices are pure 0/1.
    - Edges bucketed by (core = dst range of 12500, block = 256-dst tile,
      window = 20000-src range so gather indices fit int16) and sorted by
      dst WITHIN each bucket.  Chunks are consecutive 128-edge groups of
      the bucket (padded only to the cross-core max count, with the tail
      trimmed by trailing -1 indices), so the SWDGE descriptor count is
      minimal: the 4-queue SWDGE pipeline at ~2.4ns/descriptor is the
      kernel's hard cap, measured via a 1-queue run at exactly 4x the
      4-queue time.
    - Because a bucket's edges are dst-sorted, chunk t only touches a
      narrow dst range; the host bakes a static 128-slot column window
      (cross-core union, typically ~60 slots wide) per chunk and encodes
      slots relative to it.
  Device side (per core, one static SPMD program):
    - gpsimd: ONE dma_gather per bucket on rotating SWDGE queues.
    - DVE builds ALL one-hot tiles for a call in ONE batched
      tensor_tensor is_equal over [128, K, 130] using stride-0 broadcast
      APs (iota broadcast over chunks, per-chunk slot scalars broadcast
      over lanes).
    - PE: a zeroing matmul (zero S stream, start=1) opens each 256-block,
      then psum[feat, o_t:o_t+128] += M[e, feat].T @ S[e, 128] per chunk
      accumulates with start=0 (chunk windows overlap, so no per-chunk
      start flags; the zero-matmul makes the accumulation group well
      formed).  PSUM is one bank per block parity.
    - ACT evicts psum per block; SP DMAs the output.
  Host: transpose/concat the per-core outputs, scale rows by in_norm.
"""

from contextlib import ExitStack
from dataclasses import dataclass

import numpy as np

P = 128         # SBUF partitions / chunk size (edges per matmul)
SW = 128        # slot window width per chunk
MAX_REG = 1008  # per-gather-call descriptor budget (ucode ring is 1024)


def cdiv(a, b):
    return -(-a // b)


@dataclass(frozen=True)
class Cfg:
    n_nodes: int = 100000
    d: int = 128
    n_cores: int = 8
    blk: int = 256      # dst nodes per psum block
    win: int = 20000    # src window rows (must be < 32768 for int16 idxs)
    sfd: int = 130      # S tile free width (>= SW+1; even for alignment)
    nb_m: int = 10      # gather-destination (M tile) buffers
    nb_s: int = 6       # one-hot (S tile) buffers
    # data-dependent schedule (cross-core maxes; baked into the program)
    kk: tuple = ()      # kk[k] = chunks in call k
    cmax: tuple = ()    # cmax[k] = descriptors in call k (cross-core max)
    offs: tuple = ()    # offs[k] = per-chunk psum column offsets

    @property
    def dpc(self):
        return self.n_nodes // self.n_cores

    @property
    def nblk(self):
        return cdiv(self.dpc, self.blk)

    @property
    def n_win(self):
        return cdiv(self.n_nodes, self.win)

    @property
    def ncalls(self):
        return self.nblk * self.n_win

    @property
    def kmax(self):
        return max(self.kk)

    @property
    def nchunks(self):
        return sum(self.kk)

    @property
    def prefix(self):
        p = [0]
        for c in self.kk:
            p.append(p[-1] + c)
        return p

    @property
    def idx_cols(self):
        return self.nchunks * 8


def prep_host(u_f, v_f, src, dst, base: Cfg | None = None):
    """Bucket/sort/pad edges; returns (cfg, per-core input maps, in_norm)."""
    import ml_dtypes

    u_f = np.asarray(u_f, dtype=np.float32)
    v_f = np.asarray(v_f, dtype=np.float32)
    src = np.asarray(src).astype(np.int64)
    dst = np.asarray(dst).astype(np.int64)
    base = base or Cfg()
    N, NC, W = base.n_nodes, base.n_cores, base.n_win
    nblk = base.nblk
    ncalls = base.ncalls
    E = src.shape[0]

    node_f = np.concatenate([u_f, v_f], axis=0)
    assert node_f.shape == (N, base.d)

    deg_out = np.bincount(src, minlength=N).astype(np.float32)
    deg_in = np.bincount(dst, minlength=N).astype(np.float32)
    out_norm = np.power(np.clip(deg_out, 1.0, None), np.float32(-0.5))
    in_norm = np.power(np.clip(deg_in, 1.0, None), np.float32(-0.5))
    node_f = np.ascontiguousarray(
        (node_f * out_norm[:, None]).astype(ml_dtypes.bfloat16)
    )

    core = dst // base.dpc
    dst_loc = dst % base.dpc
    blk_id = dst_loc // base.blk
    slot256 = dst_loc % base.blk
    win_id = src // base.win
    idx16 = (src % base.win).astype(np.int16)

    k_call = blk_id * W + win_id
    bucket = core * ncalls + k_call
    counts = np.bincount(bucket, minlength=NC * ncalls).reshape(NC, ncalls)
    cmax = np.maximum(1, counts.max(axis=0))      # descs per call
    kk = cdiv(cmax, P)                            # chunks per call
    prefix = np.concatenate([[0], np.cumsum(kk)])
    nch = int(prefix[-1])

    # sort edges by (bucket, slot) so each chunk covers a narrow dst range
    so = np.lexsort((slot256, bucket))
    bs = bucket[so]
    starts = np.zeros(NC * ncalls + 1, np.int64)
    np.cumsum(counts.reshape(-1), out=starts[1:])
    offs_e = np.arange(E, dtype=np.int64) - starts[bs]
    c_of = bs // ncalls
    k_of = bs % ncalls
    pos = c_of * (nch * P) + (prefix[k_of] + offs_e // P) * P + offs_e % P

    idx_stream = np.full(NC * nch * P, -1, np.int16)
    slot_stream = np.full(NC * nch * P, -1.0, np.float32)
    idx_stream[pos] = idx16[so]
    slot_stream[pos] = slot256[so]

    # per-chunk slot window offset: cross-core union of slot spans
    sv = slot_stream.reshape(NC, nch, P)
    pad_marker = 2.0 * base.blk
    smin = np.where(sv >= 0, sv, pad_marker).min(axis=(0, 2))  # [nch]
    smax = sv.max(axis=(0, 2))                                 # [nch]
    o_t = np.clip(
        np.where(smin >= pad_marker, 0, smin), 0, base.blk - SW
    ).astype(np.int64)
    span = smax - o_t
    assert span.max() < SW, f"chunk slot span {span.max()} exceeds {SW}"
    # rebase slots to the window; pads stay -1
    slot_stream = np.where(
        slot_stream >= 0, slot_stream - np.tile(np.repeat(o_t, P), NC), -1.0
    )

    # Each core generates only its own descriptors: num_idxs_reg is loaded
    # per core from `cnt`. Rows past a core's count are never gathered
    # (their slots are -1, so the stale M rows are inert). Clamp to >= 1
    # and make row 0 a valid index for buckets empty on some core.
    # Each core generates only its own descriptors: num_idxs_reg is read
    # per core from `cnt` (rows past a core's count stay -1 and are never
    # gathered; their slots are -1 so stale M rows are inert). For buckets
    # empty on a core, clamp to 1 descriptor and make row 0 a valid index.
    cnts = np.maximum(1, counts).astype(np.int32)
    for c, k in np.argwhere(counts == 0):
        idx_stream[c * (nch * P) + prefix[k] * P] = 0

    offs_cfg = []
    for k in range(ncalls):
        offs_cfg.append(tuple(int(x) for x in o_t[prefix[k]:prefix[k + 1]]))

    cfg = Cfg(
        n_nodes=base.n_nodes, d=base.d, n_cores=base.n_cores, blk=base.blk,
        win=base.win, sfd=base.sfd, nb_m=base.nb_m, nb_s=base.nb_s,
        kk=tuple(int(x) for x in kk), cmax=tuple(int(x) for x in cmax),
        offs=tuple(offs_cfg),
    )
    assert max(cfg.cmax) <= MAX_REG, max(cfg.cmax)

    in_maps = []
    for c in range(NC):
        seg = slice(c * nch * P, (c + 1) * nch * P)
        xi = idx_stream[seg].reshape(nch * 8, 16)
        xi = np.ascontiguousarray(np.tile(xi.T, (8, 1)))
        sl = np.ascontiguousarray(
            slot_stream[seg].reshape(nch, P).T.astype(ml_dtypes.bfloat16)
        )
        cn = np.ascontiguousarray(np.tile(cnts[c][None, :], (P, 1)))
        in_maps.append({"nf": node_f, "idx": xi, "slots": sl, "cnt": cn})
    return cfg, in_maps, in_norm


def build_nc(cfg: Cfg):
    import concourse.bacc as bacc
    import concourse.mybir as mybir
    from concourse.ap import AP
    from concourse.library_config import mlp

    f32 = mybir.dt.float32
    bf16 = mybir.dt.bfloat16
    AF = mybir.ActivationFunctionType
    D, W, nblk = cfg.d, cfg.n_win, cfg.nblk
    ncalls, nchunks, kmax = cfg.ncalls, cfg.nchunks, cfg.kmax
    prefix = cfg.prefix
    idx_cols = cfg.idx_cols
    colpre = [p * 8 for p in prefix]

    nc = bacc.Bacc(
        "TRN2", target_bir_lowering=False, num_swdge_queues=4,
        dynamic_dma_scratch_size=49152,
    )

    nf = nc.dram_tensor("nf", [cfg.n_nodes, D], bf16, kind="ExternalInput")
    idx_d = nc.dram_tensor("idx", [P, idx_cols], mybir.dt.int16, kind="ExternalInput")
    slots_d = nc.dram_tensor("slots", [P, nchunks], bf16, kind="ExternalInput")
    cnt_d = nc.dram_tensor("cnt", [P, ncalls], mybir.dt.int32, kind="ExternalInput")
    out_d = nc.dram_tensor("out", [P, nblk * cfg.blk], f32, kind="ExternalOutput")

    with ExitStack() as ctx:
        ec = ctx.enter_context
        idx_sb = ec(nc.sbuf_tensor("idx_sb", [P, idx_cols], mybir.dt.int16))
        slots_sb = ec(nc.sbuf_tensor("slots_sb", [P, nchunks], bf16))
        cnt_sb = ec(nc.sbuf_tensor("cnt_sb", [P, ncalls], mybir.dt.int32))
        iota_sb = ec(nc.sbuf_tensor("iota_sb", [P, cfg.sfd], bf16))
        szero = ec(nc.sbuf_tensor("szero", [P, cfg.blk], bf16))
        m_sbs = [ec(nc.sbuf_tensor(f"m{j}", [P, kmax, D], bf16)) for j in range(cfg.nb_m)]
        s_sbs = [ec(nc.sbuf_tensor(f"s{j}", [P, kmax, cfg.sfd], bf16)) for j in range(cfg.nb_s)]
        obufs = [ec(nc.sbuf_tensor(f"ob{j}", [P, cfg.blk], f32)) for j in range(2)]
        psums = [ec(nc.psum_tensor(f"ps{j}", [P, cfg.blk], f32)) for j in range(2)]

        iosl = ec(nc.semaphore("iosl"))
        iocn = ec(nc.semaphore("iocn"))
        ioix = [ec(nc.semaphore(f"ioix{p}")) for p in range(4)]
        init = ec(nc.semaphore("init"))
        gsems = [ec(nc.semaphore(f"gat{j}")) for j in range(cfg.nb_m)]
        sv = ec(nc.semaphore("sv"))
        pe = ec(nc.semaphore("pe"))
        ev = ec(nc.semaphore("ev"))
        osems = [ec(nc.semaphore(f"odma{j}")) for j in range(2)]

        with nc.Block() as block:

            @block.sync
            def _(sync):
                sync.dma_start(slots_sb[:], slots_d[:]).then_inc(iosl, 16)
                sync.dma_start(cnt_sb[:], cnt_d[:]).then_inc(iocn, 16)
                qc = idx_cols // 4
                for piece in range(4):
                    lo = piece * qc
                    hi = idx_cols if piece == 3 else (piece + 1) * qc
                    sync.dma_start(
                        idx_sb[:, lo:hi], idx_d[:, lo:hi]
                    ).then_inc(ioix[piece], 16)
                for b in range(nblk):
                    sync.wait_ge(ev, b + 1)
                    sync.dma_start(
                        out_d[:, b * cfg.blk:(b + 1) * cfg.blk], obufs[b % 2][:]
                    ).then_inc(osems[b % 2], 16)
                sync.wait_ge(osems[0], 16 * cdiv(nblk, 2))
                if nblk > 1:
                    sync.wait_ge(osems[1], 16 * (nblk // 2))

            @block.gpsimd
            def _(g):
                g.iota(
                    iota_sb[:], [[1, cfg.sfd]], channel_multiplier=0,
                    allow_small_or_imprecise_dtypes=True,
                ).then_inc(init, 1)
                g.memset(szero[:], 0).then_inc(init, 1)
                for j in range(cfg.nb_m):
                    g.memset(m_sbs[j][:], 0).then_inc(init, 1)
                g.load_library(mlp)
                g.wait_ge(init, 2 + cfg.nb_m)
                g.wait_ge(iocn, 16)
                cnt_reg = g.alloc_register("cnt_reg")
                qc = idx_cols // 4
                pc_seen = -1
                for k in range(ncalls):
                    w = k % W
                    end_col = colpre[k + 1]
                    piece = 3 if end_col > 3 * qc else (end_col - 1) // qc
                    while piece > pc_seen:
                        pc_seen += 1
                        g.wait_ge(ioix[pc_seen], 16)
                    if k >= cfg.nb_m:
                        g.wait_ge(pe, prefix[k - cfg.nb_m + 1] + nblk_before(k - cfg.nb_m + 1, W))
                    rows = min(cfg.win, cfg.n_nodes - w * cfg.win)
                    j = k % cfg.nb_m
                    src_v = nf[w * cfg.win: w * cfg.win + rows, :]
                    g.reg_load(cnt_reg, cnt_sb[0:1, k:k + 1])
                    g.dma_gather(
                        m_sbs[j][:, 0:cfg.kk[k], :],
                        src_v,
                        idx_sb[:, colpre[k]:colpre[k + 1]],
                        cfg.kk[k] * P,
                        cnt_reg,
                        D,
                        queue_num=k % 4,
                    ).then_inc(gsems[j], 16)

            @block.vector
            def _(v):
                v.wait_ge(iosl, 16)
                v.wait_ge(init, 1)
                for k in range(ncalls):
                    if k >= cfg.nb_s:
                        v.wait_ge(pe, prefix[k - cfg.nb_s + 1] + nblk_before(k - cfg.nb_s + 1, W))
                    K = cfg.kk[k]
                    jb = k % cfg.nb_s
                    o = s_sbs[jb][:, 0:K, :]
                    a = iota_sb[:]
                    in0 = AP(a.tensor, a.offset, [list(a.ap[0]), [0, K], list(a.ap[1])])
                    b = slots_sb[:, prefix[k]:prefix[k] + K]
                    in1 = AP(b.tensor, b.offset, [list(b.ap[0]), list(b.ap[1]), [0, cfg.sfd]])
                    v.tensor_tensor(
                        o, in0, in1, mybir.AluOpType.is_equal
                    ).then_inc(sv, 1)

            @block.scalar
            def _(a):
                for b in range(nblk):
                    a.wait_ge(pe, prefix[(b + 1) * W] + (b + 1))
                    if b >= 2:
                        a.wait_ge(osems[b % 2], 16 * (b // 2))
                    a.activation(
                        obufs[b % 2][:], psums[b % 2][:], AF.Copy,
                    ).then_inc(ev, 1)

            @block.tensor
            def _(te):
                for b in range(nblk):
                    for w in range(W):
                        k = b * W + w
                        j = k % cfg.nb_m
                        te.wait_ge(gsems[j], 16 * (k // cfg.nb_m + 1))
                        te.wait_ge(sv, k + 1)
                        if w == 0:
                            if b >= 2:
                                te.wait_ge(ev, b - 1)
                            # zeroing matmul opens the block's accum group
                            te.matmul(
                                psums[b % 2][:],
                                m_sbs[j][:, 0, :],
                                szero[:],
                                start=True,
                                stop=False,
                            ).then_inc(pe, 1)
                        last_call = (w == W - 1)
                        for i in range(cfg.kk[k]):
                            o = cfg.offs[k][i]
                            te.matmul(
                                psums[b % 2][:, o:o + SW],
                                m_sbs[j][:, i, :],
                                s_sbs[k % cfg.nb_s][:, i, 0:SW],
                                start=False,
                                stop=last_call and i == cfg.kk[k] - 1,
                            ).then_inc(pe, 1)

    nc.compile()
    return nc


def nblk_before(ncall, W):
    """Number of block-zeroing matmuls issued before call `ncall` starts."""
    return (ncall + W - 1) // W


def unshard(cfg: Cfg, results, in_norm):
    out = np.empty((cfg.n_nodes, cfg.d), np.float32)
    for c in range(cfg.n_cores):
        o = results[c]["out"]
        out[c * cfg.dpc:(c + 1) * cfg.dpc, :] = o[:, :cfg.dpc].T
    out *= in_norm[:, None]
    return out


def run(inputs, trace=False, **spmd_kwargs):
    from concourse.bass_utils import run_bass_kernel_spmd

    cfg, in_maps, in_norm = prep_host(
        inputs["u_f"], inputs["v_f"], inputs["src"], inputs["dst"]
    )
    nc = build_nc(cfg)
    res = run_bass_kernel_spmd(
        nc, in_maps, core_ids=list(range(cfg.n_cores)), trace=trace,
        **spmd_kwargs,
    )
    return unshard(cfg, res.results, in_norm), res


def kernel(**inputs):
    return run(inputs)[0]


# revision 19
# speedup vs baseline: 1.1908x; 1.1908x over previous
"""GCN message-passing layer (copy_src -> segment_sum -> dual degree norm)
on 8 Trainium2 NeuronCores.

Strategy (dst-sharded message passing, v3):
  Host side (sharding/metadata only):
    - node_f = concat(u_f, v_f) * out_norm[src-side], cast to bf16.
      in-degree norm is applied to the FINAL output on the host (a
      per-dst-row scale commutes with the edge aggregation), so on-device
      one-hot matrices are pure 0/1.
    - Edges bucketed by (core = dst range of 12500, block = 256-dst tile,
      window = 20000-src range so gather indices fit int16) and sorted by
      dst WITHIN each bucket.  Chunks are consecutive 128-edge groups of
      the bucket (padded only to the cross-core max count, with the tail
      trimmed by trailing -1 indices), so the SWDGE descriptor count is
      minimal: the 4-queue SWDGE pipeline at ~2.4ns/descriptor is the
      kernel's hard cap, measured via a 1-queue run at exactly 4x the
      4-queue time.
    - Because a bucket's edges are dst-sorted, chunk t only touches a
      narrow dst range; the host bakes a static 128-slot column window
      (cross-core union, typically ~60 slots wide) per chunk and encodes
      slots relative to it.
  Device side (per core, one static SPMD program):
    - gpsimd: ONE dma_gather per bucket on rotating SWDGE queues.
    - DVE builds ALL one-hot tiles for a call in ONE batched
      tensor_tensor is_equal over [128, K, 130] using stride-0 broadcast
      APs (iota broadcast over chunks, per-chunk slot scalars broadcast
      over lanes).
    - PE: a zeroing matmul (zero S stream, start=1) opens each 256-block,
      then psum[feat, o_t:o_t+128] += M[e, feat].T @ S[e, 128] per chunk
      accumulates with start=0 (chunk windows overlap, so no per-chunk
      start flags; the zero-matmul makes the accumulation group well
      formed).  PSUM is one bank per block parity.
    - ACT evicts psum per block; SP DMAs the output.
  Host: transpose/concat the per-core outputs, scale rows by in_norm.
"""

from contextlib import ExitStack
from dataclasses import dataclass

import numpy as np

P = 128         # SBUF partitions / chunk size (edges per matmul)
SW = 128        # slot window width per chunk
MAX_REG = 1008  # per-gather-call descriptor budget (ucode ring is 1024)


def cdiv(a, b):
    return -(-a // b)


@dataclass(frozen=True)
class Cfg:
    n_nodes: int = 100000
    d: int = 128
    n_cores: int = 8
    blk: int = 256      # dst nodes per psum block
    win: int = 20000    # src window rows (must be < 32768 for int16 idxs)
    sfd: int = 130      # S tile free width (>= SW+1; even for alignment)
    nb_m: int = 10      # gather-destination (M tile) buffers
    nb_s: int = 6       # one-hot (S tile) buffers
    # data-dependent schedule (cross-core maxes; baked into the program)
    kk: tuple = ()      # kk[k] = chunks in call k
    cmax: tuple = ()    # cmax[k] = descriptors in call k (cross-core max)
    offs: tuple = ()    # offs[k] = per-chunk psum column offsets

    @property
    def dpc(self):
        return self.n_nodes // self.n_cores

    @property
    def nblk(self):
        return cdiv(self.dpc, self.blk)

    @property
    def n_win(self):
        return cdiv(self.n_nodes, self.win)

    @property
    def ncalls(self):
        return self.nblk * self.n_win

    @property
    def kmax(self):
        return max(self.kk)

    @property
    def nchunks(self):
        return sum(self.kk)

    @property
    def prefix(self):
        p = [0]
        for c in self.kk:
            p.append(p[-1] + c)
        return p

    @property
    def idx_cols(self):
        return self.nchunks * 8


def prep_host(u_f, v_f, src, dst, base: Cfg | None = None):
    """Bucket/sort/pad edges; returns (cfg, per-core input maps, in_norm)."""
    import ml_dtypes

    u_f = np.asarray(u_f, dtype=np.float32)
    v_f = np.asarray(v_f, dtype=np.float32)
    src = np.asarray(src).astype(np.int64)
    dst = np.asarray(dst).astype(np.int64)
    base = base or Cfg()
    N, NC, W = base.n_nodes, base.n_cores, base.n_win
    nblk = base.nblk
    ncalls = base.ncalls
    E = src.shape[0]

    node_f = np.concatenate([u_f, v_f], axis=0)
    assert node_f.shape == (N, base.d)

    deg_out = np.bincount(src, minlength=N).astype(np.float32)
    deg_in = np.bincount(dst, minlength=N).astype(np.float32)
    out_norm = np.power(np.clip(deg_out, 1.0, None), np.float32(-0.5))
    in_norm = np.power(np.clip(deg_in, 1.0, None), np.float32(-0.5))
    node_f = np.ascontiguousarray(
        (node_f * out_norm[:, None]).astype(ml_dtypes.bfloat16)
    )

    core = dst // base.dpc
    dst_loc = dst % base.dpc
    blk_id = dst_loc // base.blk
    slot256 = dst_loc % base.blk
    win_id = src // base.win
    idx16 = (src % base.win).astype(np.int16)

    k_call = blk_id * W + win_id
    bucket = core * ncalls + k_call
    counts = np.bincount(bucket, minlength=NC * ncalls).reshape(NC, ncalls)
    cmax = np.maximum(1, counts.max(axis=0))      # descs per call
    kk = cdiv(cmax, P)                            # chunks per call
    prefix = np.concatenate([[0], np.cumsum(kk)])
    nch = int(prefix[-1])

    # sort edges by (bucket, slot) so each chunk covers a narrow dst range
    so = np.lexsort((slot256, bucket))
    bs = bucket[so]
    starts = np.zeros(NC * ncalls + 1, np.int64)
    np.cumsum(counts.reshape(-1), out=starts[1:])
    offs_e = np.arange(E, dtype=np.int64) - starts[bs]
    c_of = bs // ncalls
    k_of = bs % ncalls
    pos = c_of * (nch * P) + (prefix[k_of] + offs_e // P) * P + offs_e % P

    idx_stream = np.full(NC * nch * P, -1, np.int16)
    slot_stream = np.full(NC * nch * P, -1.0, np.float32)
    idx_stream[pos] = idx16[so]
    slot_stream[pos] = slot256[so]

    # per-chunk slot window offset: cross-core union of slot spans
    sv = slot_stream.reshape(NC, nch, P)
    pad_marker = 2.0 * base.blk
    smin = np.where(sv >= 0, sv, pad_marker).min(axis=(0, 2))  # [nch]
    smax = sv.max(axis=(0, 2))                                 # [nch]
    o_t = np.clip(
        np.where(smin >= pad_marker, 0, smin), 0, base.blk - SW
    ).astype(np.int64)
    span = smax - o_t
    assert span.max() < SW, f"chunk slot span {span.max()} exceeds {SW}"
    # rebase slots to the window; pads stay -1
    slot_stream = np.where(
        slot_stream >= 0, slot_stream - np.tile(np.repeat(o_t, P), NC), -1.0
    )

    # pad gathered rows (count_c..cmax) with index 0 (slot stays -1):
    # the shared num_idxs_reg must match every core's post-trim
    # descriptor count exactly
    for c in range(NC):
        cbase = c * (nch * P)
        for k in range(ncalls):
            n0 = int(counts[c, k])
            n1 = int(cmax[k])
            if n0 < n1:
                st = cbase + prefix[k] * P
                idx_stream[st + n0: st + n1] = 0

    offs_cfg = []
    for k in range(ncalls):
        offs_cfg.append(tuple(int(x) for x in o_t[prefix[k]:prefix[k + 1]]))

    cfg = Cfg(
        n_nodes=base.n_nodes, d=base.d, n_cores=base.n_cores, blk=base.blk,
        win=base.win, sfd=base.sfd, nb_m=base.nb_m, nb_s=base.nb_s,
        kk=tuple(int(x) for x in kk), cmax=tuple(int(x) for x in cmax),
        offs=tuple(offs_cfg),
    )
    assert max(cfg.cmax) <= MAX_REG, max(cfg.cmax)

    in_maps = []
    for c in range(NC):
        seg = slice(c * nch * P, (c + 1) * nch * P)
        xi = idx_stream[seg].reshape(nch * 8, 16)
        xi = np.ascontiguousarray(np.tile(xi.T, (8, 1)))
        sl = np.ascontiguousarray(
            slot_stream[seg].reshape(nch, P).T.astype(ml_dtypes.bfloat16)
        )
        in_maps.append({"nf": node_f, "idx": xi, "slots": sl})
    return cfg, in_maps, in_norm


def build_nc(cfg: Cfg):
    import concourse.bacc as bacc
    import concourse.mybir as mybir
    from concourse.ap import AP
    from concourse.library_config import mlp

    f32 = mybir.dt.float32
    bf16 = mybir.dt.bfloat16
    AF = mybir.ActivationFunctionType
    D, W, nblk = cfg.d, cfg.n_win, cfg.nblk
    ncalls, nchunks, kmax = cfg.ncalls, cfg.nchunks, cfg.kmax
    prefix = cfg.prefix
    idx_cols = cfg.idx_cols
    colpre = [p * 8 for p in prefix]

    nc = bacc.Bacc(
        "TRN2", target_bir_lowering=False, num_swdge_queues=4,
        dynamic_dma_scratch_size=49152,
    )

    nf = nc.dram_tensor("nf", [cfg.n_nodes, D], bf16, kind="ExternalInput")
    idx_d = nc.dram_tensor("idx", [P, idx_cols], mybir.dt.int16, kind="ExternalInput")
    slots_d = nc.dram_tensor("slots", [P, nchunks], bf16, kind="ExternalInput")
    out_d = nc.dram_tensor("out", [P, nblk * cfg.blk], f32, kind="ExternalOutput")

    with ExitStack() as ctx:
        ec = ctx.enter_context
        idx_sb = ec(nc.sbuf_tensor("idx_sb", [P, idx_cols], mybir.dt.int16))
        slots_sb = ec(nc.sbuf_tensor("slots_sb", [P, nchunks], bf16))
        iota_sb = ec(nc.sbuf_tensor("iota_sb", [P, cfg.sfd], bf16))
        szero = ec(nc.sbuf_tensor("szero", [P, cfg.blk], bf16))
        m_sbs = [ec(nc.sbuf_tensor(f"m{j}", [P, kmax, D], bf16)) for j in range(cfg.nb_m)]
        s_sbs = [ec(nc.sbuf_tensor(f"s{j}", [P, kmax, cfg.sfd], bf16)) for j in range(cfg.nb_s)]
        obufs = [ec(nc.sbuf_tensor(f"ob{j}", [P, cfg.blk], f32)) for j in range(2)]
        psums = [ec(nc.psum_tensor(f"ps{j}", [P, cfg.blk], f32)) for j in range(2)]

        iosl = ec(nc.semaphore("iosl"))
        ioix = [ec(nc.semaphore(f"ioix{p}")) for p in range(4)]
        init = ec(nc.semaphore("init"))
        gsems = [ec(nc.semaphore(f"gat{j}")) for j in range(cfg.nb_m)]
        sv = ec(nc.semaphore("sv"))
        pe = ec(nc.semaphore("pe"))
        ev = ec(nc.semaphore("ev"))
        osems = [ec(nc.semaphore(f"odma{j}")) for j in range(2)]

        with nc.Block() as block:

            @block.sync
            def _(sync):
                sync.dma_start(slots_sb[:], slots_d[:]).then_inc(iosl, 16)
                qc = idx_cols // 4
                for piece in range(4):
                    lo = piece * qc
                    hi = idx_cols if piece == 3 else (piece + 1) * qc
                    sync.dma_start(
                        idx_sb[:, lo:hi], idx_d[:, lo:hi]
                    ).then_inc(ioix[piece], 16)
                for b in range(nblk):
                    sync.wait_ge(ev, b + 1)
                    sync.dma_start(
                        out_d[:, b * cfg.blk:(b + 1) * cfg.blk], obufs[b % 2][:]
                    ).then_inc(osems[b % 2], 16)
                sync.wait_ge(osems[0], 16 * cdiv(nblk, 2))
                if nblk > 1:
                    sync.wait_ge(osems[1], 16 * (nblk // 2))

            @block.gpsimd
            def _(g):
                g.iota(
                    iota_sb[:], [[1, cfg.sfd]], channel_multiplier=0,
                    allow_small_or_imprecise_dtypes=True,
                ).then_inc(init, 1)
                g.memset(szero[:], 0).then_inc(init, 1)
                for j in range(cfg.nb_m):
                    g.memset(m_sbs[j][:], 0).then_inc(init, 1)
                g.load_library(mlp)
                g.wait_ge(init, 2 + cfg.nb_m)
                qc = idx_cols // 4
                pc_seen = -1
                for k in range(ncalls):
                    w = k % W
                    end_col = colpre[k + 1]
                    piece = 3 if end_col > 3 * qc else (end_col - 1) // qc
                    while piece > pc_seen:
                        pc_seen += 1
                        g.wait_ge(ioix[pc_seen], 16)
                    if k >= cfg.nb_m:
                        g.wait_ge(pe, prefix[k - cfg.nb_m + 1] + nblk_before(k - cfg.nb_m + 1, W))
                    rows = min(cfg.win, cfg.n_nodes - w * cfg.win)
                    j = k % cfg.nb_m
                    src_v = nf[w * cfg.win: w * cfg.win + rows, :]
                    g.dma_gather(
                        m_sbs[j][:, 0:cfg.kk[k], :],
                        src_v,
                        idx_sb[:, colpre[k]:colpre[k + 1]],
                        cfg.kk[k] * P,
                        cfg.cmax[k],
                        D,
                        queue_num=k % 4,
                    ).then_inc(gsems[j], 16)

            @block.vector
            def _(v):
                v.wait_ge(iosl, 16)
                v.wait_ge(init, 1)
                for k in range(ncalls):
                    if k >= cfg.nb_s:
                        v.wait_ge(pe, prefix[k - cfg.nb_s + 1] + nblk_before(k - cfg.nb_s + 1, W))
                    K = cfg.kk[k]
                    jb = k % cfg.nb_s
                    o = s_sbs[jb][:, 0:K, :]
                    a = iota_sb[:]
                    in0 = AP(a.tensor, a.offset, [list(a.ap[0]), [0, K], list(a.ap[1])])
                    b = slots_sb[:, prefix[k]:prefix[k] + K]
                    in1 = AP(b.tensor, b.offset, [list(b.ap[0]), list(b.ap[1]), [0, cfg.sfd]])
                    v.tensor_tensor(
                        o, in0, in1, mybir.AluOpType.is_equal
                    ).then_inc(sv, 1)

            @block.scalar
            def _(a):
                for b in range(nblk):
                    a.wait_ge(pe, prefix[(b + 1) * W] + (b + 1))
                    if b >= 2:
                        a.wait_ge(osems[b % 2], 16 * (b // 2))
                    a.activation(
                        obufs[b % 2][:], psums[b % 2][:], AF.Copy,
                    ).then_inc(ev, 1)

            @block.tensor
            def _(te):
                for b in range(nblk):
                    for w in range(W):
                        k = b * W + w
                        j = k % cfg.nb_m
                        te.wait_ge(gsems[j], 16 * (k // cfg.nb_m + 1))
                        te.wait_ge(sv, k + 1)
                        if w == 0:
                            if b >= 2:
                                te.wait_ge(ev, b - 1)
                            # zeroing matmul opens the block's accum group
                            te.matmul(
                                psums[b % 2][:],
                                m_sbs[j][:, 0, :],
                                szero[:],
                                start=True,
                                stop=False,
                            ).then_inc(pe, 1)
                        last_call = (w == W - 1)
                        for i in range(cfg.kk[k]):
                            o = cfg.offs[k][i]
                            te.matmul(
                                psums[b % 2][:, o:o + SW],
                                m_sbs[j][:, i, :],
                                s_sbs[k % cfg.nb_s][:, i, 0:SW],
                                start=False,
                                stop=last_call and i == cfg.kk[k] - 1,
                            ).then_inc(pe, 1)

    nc.compile()
    return nc


def nblk_before(ncall, W):
    """Number of block-zeroing matmuls issued before call `ncall` starts."""
    return (ncall + W - 1) // W


def unshard(cfg: Cfg, results, in_norm):
    out = np.empty((cfg.n_nodes, cfg.d), np.float32)
    for c in range(cfg.n_cores):
        o = results[c]["out"]
        out[c * cfg.dpc:(c + 1) * cfg.dpc, :] = o[:, :cfg.dpc].T
    out *= in_norm[:, None]
    return out


def run(inputs, trace=False, **spmd_kwargs):
    from concourse.bass_utils import run_bass_kernel_spmd

    cfg, in_maps, in_norm = prep_host(
        inputs["u_f"], inputs["v_f"], inputs["src"], inputs["dst"]
    )
    nc = build_nc(cfg)
    res = run_bass_kernel_spmd(
        nc, in_maps, core_ids=list(range(cfg.n_cores)), trace=trace,
        **spmd_kwargs,
    )
    return unshard(cfg, res.results, in_norm), res


def kernel(**inputs):
    return run(inputs)[0]


# revision 21
# speedup vs baseline: 1.1993x; 1.0071x over previous
"""GCN message-passing layer (copy_src -> segment_sum -> dual degree norm)
on 8 Trainium2 NeuronCores.

Strategy (dst-sharded message passing, v3):
  Host side (sharding/metadata only):
    - node_f = concat(u_f, v_f) * out_norm[src-side], cast to bf16.
      in-degree norm is applied to the FINAL output on the host (a
      per-dst-row scale commutes with the edge aggregation), so on-device
      one-hot matrices are pure 0/1.
    - Edges bucketed by (core = dst range of 12500, block = 256-dst tile,
      window = 20000-src range so gather indices fit int16) and sorted by
      dst WITHIN each bucket.  Chunks are consecutive 128-edge groups of
      the bucket (padded only to the cross-core max count, with the tail
      trimmed by trailing -1 indices), so the SWDGE descriptor count is
      minimal: the 4-queue SWDGE pipeline at ~2.4ns/descriptor is the
      kernel's hard cap, measured via a 1-queue run at exactly 4x the
      4-queue time.
    - Because a bucket's edges are dst-sorted, chunk t only touches a
      narrow dst range; the host bakes a static 128-slot column window
      (cross-core union, typically ~60 slots wide) per chunk and encodes
      slots relative to it.
  Device side (per core, one static SPMD program):
    - gpsimd: ONE dma_gather per bucket on rotating SWDGE queues.
    - DVE builds ALL one-hot tiles for a call in ONE batched
      tensor_tensor is_equal over [128, K, 130] using stride-0 broadcast
      APs (iota broadcast over chunks, per-chunk slot scalars broadcast
      over lanes).
    - PE: a zeroing matmul (zero S stream, start=1) opens each 256-block,
      then psum[feat, o_t:o_t+128] += M[e, feat].T @ S[e, 128] per chunk
      accumulates with start=0 (chunk windows overlap, so no per-chunk
      start flags; the zero-matmul makes the accumulation group well
      formed).  PSUM is one bank per block parity.
    - ACT evicts psum per block; SP DMAs the output.
  Host: transpose/concat the per-core outputs, scale rows by in_norm.
"""

from contextlib import ExitStack
from dataclasses import dataclass

import numpy as np

P = 128         # SBUF partitions / chunk size (edges per matmul)
SW = 128        # slot window width per chunk
MAX_REG = 1008  # per-gather-call descriptor budget (ucode ring is 1024)


def cdiv(a, b):
    return -(-a // b)


@dataclass(frozen=True)
class Cfg:
    n_nodes: int = 100000
    d: int = 128
    n_cores: int = 8
    blk: int = 256      # dst nodes per psum block
    win: int = 20000    # src window rows (must be < 32768 for int16 idxs)
    sfd: int = 130      # S tile free width (>= SW+1; even for alignment)
    nb_m: int = 10      # gather-destination (M tile) buffers
    nb_s: int = 6       # one-hot (S tile) buffers
    # data-dependent schedule (cross-core maxes; baked into the program)
    kk: tuple = ()      # kk[k] = chunks in call k
    cmax: tuple = ()    # cmax[k] = descriptors in call k (cross-core max)
    offs: tuple = ()    # offs[k] = per-chunk psum column offsets

    @property
    def dpc(self):
        return self.n_nodes // self.n_cores

    @property
    def nblk(self):
        return cdiv(self.dpc, self.blk)

    @property
    def n_win(self):
        return cdiv(self.n_nodes, self.win)

    @property
    def ncalls(self):
        return self.nblk * self.n_win

    @property
    def kmax(self):
        return max(self.kk)

    @property
    def nchunks(self):
        return sum(self.kk)

    @property
    def prefix(self):
        p = [0]
        for c in self.kk:
            p.append(p[-1] + c)
        return p

    @property
    def idx_cols(self):
        return self.nchunks * 8


def prep_host(u_f, v_f, src, dst, base: Cfg | None = None):
    """Bucket/sort/pad edges; returns (cfg, per-core input maps, in_norm)."""
    import ml_dtypes

    u_f = np.asarray(u_f, dtype=np.float32)
    v_f = np.asarray(v_f, dtype=np.float32)
    src = np.asarray(src).astype(np.int64)
    dst = np.asarray(dst).astype(np.int64)
    base = base or Cfg()
    N, NC, W = base.n_nodes, base.n_cores, base.n_win
    nblk = base.nblk
    ncalls = base.ncalls
    E = src.shape[0]

    node_f = np.concatenate([u_f, v_f], axis=0)
    assert node_f.shape == (N, base.d)

    deg_out = np.bincount(src, minlength=N).astype(np.float32)
    deg_in = np.bincount(dst, minlength=N).astype(np.float32)
    out_norm = np.power(np.clip(deg_out, 1.0, None), np.float32(-0.5))
    in_norm = np.power(np.clip(deg_in, 1.0, None), np.float32(-0.5))
    node_f = np.ascontiguousarray(
        (node_f * out_norm[:, None]).astype(ml_dtypes.bfloat16)
    )

    core = dst // base.dpc
    dst_loc = dst % base.dpc
    blk_id = dst_loc // base.blk
    slot256 = dst_loc % base.blk
    win_id = src // base.win
    idx16 = (src % base.win).astype(np.int16)

    k_call = blk_id * W + win_id
    bucket = core * ncalls + k_call
    counts = np.bincount(bucket, minlength=NC * ncalls).reshape(NC, ncalls)
    cmax = np.maximum(1, counts.max(axis=0))      # descs per call
    kk = cdiv(cmax, P)                            # chunks per call
    prefix = np.concatenate([[0], np.cumsum(kk)])
    nch = int(prefix[-1])

    # sort edges by (bucket, slot) so each chunk covers a narrow dst range
    so = np.lexsort((slot256, bucket))
    bs = bucket[so]
    starts = np.zeros(NC * ncalls + 1, np.int64)
    np.cumsum(counts.reshape(-1), out=starts[1:])
    offs_e = np.arange(E, dtype=np.int64) - starts[bs]
    c_of = bs // ncalls
    k_of = bs % ncalls
    pos = c_of * (nch * P) + (prefix[k_of] + offs_e // P) * P + offs_e % P

    idx_stream = np.full(NC * nch * P, -1, np.int16)
    slot_stream = np.full(NC * nch * P, -1.0, np.float32)
    idx_stream[pos] = idx16[so]
    slot_stream[pos] = slot256[so]

    # per-chunk slot window offset: cross-core union of slot spans
    sv = slot_stream.reshape(NC, nch, P)
    pad_marker = 2.0 * base.blk
    smin = np.where(sv >= 0, sv, pad_marker).min(axis=(0, 2))  # [nch]
    smax = sv.max(axis=(0, 2))                                 # [nch]
    o_t = np.clip(
        np.where(smin >= pad_marker, 0, smin), 0, base.blk - SW
    ).astype(np.int64)
    span = smax - o_t
    assert span.max() < SW, f"chunk slot span {span.max()} exceeds {SW}"
    # rebase slots to the window; pads stay -1
    slot_stream = np.where(
        slot_stream >= 0, slot_stream - np.tile(np.repeat(o_t, P), NC), -1.0
    )

    # pad gathered rows (count_c..cmax) with index 0 (slot stays -1):
    # the shared num_idxs_reg must match every core's post-trim
    # descriptor count exactly
    for c in range(NC):
        cbase = c * (nch * P)
        for k in range(ncalls):
            n0 = int(counts[c, k])
            n1 = int(cmax[k])
            if n0 < n1:
                st = cbase + prefix[k] * P
                idx_stream[st + n0: st + n1] = 0

    offs_cfg = []
    for k in range(ncalls):
        offs_cfg.append(tuple(int(x) for x in o_t[prefix[k]:prefix[k + 1]]))

    cfg = Cfg(
        n_nodes=base.n_nodes, d=base.d, n_cores=base.n_cores, blk=base.blk,
        win=base.win, sfd=base.sfd, nb_m=base.nb_m, nb_s=base.nb_s,
        kk=tuple(int(x) for x in kk), cmax=tuple(int(x) for x in cmax),
        offs=tuple(offs_cfg),
    )
    assert max(cfg.cmax) <= MAX_REG, max(cfg.cmax)

    in_maps = []
    for c in range(NC):
        seg = slice(c * nch * P, (c + 1) * nch * P)
        xi = idx_stream[seg].reshape(nch * 8, 16)
        xi = np.ascontiguousarray(np.tile(xi.T, (8, 1)))
        sl = np.ascontiguousarray(
            slot_stream[seg].reshape(nch, P).T.astype(ml_dtypes.bfloat16)
        )
        in_maps.append({"nf": node_f, "idx": xi, "slots": sl})
    return cfg, in_maps, in_norm


def build_nc(cfg: Cfg):
    import concourse.bacc as bacc
    import concourse.mybir as mybir
    from concourse.ap import AP
    from concourse.library_config import mlp

    f32 = mybir.dt.float32
    bf16 = mybir.dt.bfloat16
    AF = mybir.ActivationFunctionType
    D, W, nblk = cfg.d, cfg.n_win, cfg.nblk
    ncalls, nchunks, kmax = cfg.ncalls, cfg.nchunks, cfg.kmax
    prefix = cfg.prefix
    idx_cols = cfg.idx_cols
    colpre = [p * 8 for p in prefix]

    nc = bacc.Bacc(
        "TRN2", target_bir_lowering=False, num_swdge_queues=4,
        dynamic_dma_scratch_size=49152,
    )

    nf = nc.dram_tensor("nf", [cfg.n_nodes, D], bf16, kind="ExternalInput")
    idx_d = nc.dram_tensor("idx", [P, idx_cols], mybir.dt.int16, kind="ExternalInput")
    slots_d = nc.dram_tensor("slots", [P, nchunks], bf16, kind="ExternalInput")
    out_d = nc.dram_tensor("out", [P, nblk * cfg.blk], f32, kind="ExternalOutput")

    with ExitStack() as ctx:
        ec = ctx.enter_context
        idx_sb = ec(nc.sbuf_tensor("idx_sb", [P, idx_cols], mybir.dt.int16))
        slots_sb = ec(nc.sbuf_tensor("slots_sb", [P, nchunks], bf16))
        iota_sb = ec(nc.sbuf_tensor("iota_sb", [P, cfg.sfd], bf16))
        szero = ec(nc.sbuf_tensor("szero", [P, cfg.blk], bf16))
        m_sbs = [ec(nc.sbuf_tensor(f"m{j}", [P, kmax, D], bf16)) for j in range(cfg.nb_m)]
        s_sbs = [ec(nc.sbuf_tensor(f"s{j}", [P, kmax, cfg.sfd], bf16)) for j in range(cfg.nb_s)]
        obufs = [ec(nc.sbuf_tensor(f"ob{j}", [P, cfg.blk], f32)) for j in range(2)]
        psums = [ec(nc.psum_tensor(f"ps{j}", [P, cfg.blk], f32)) for j in range(2)]

        iosl = ec(nc.semaphore("iosl"))
        ioix = [ec(nc.semaphore(f"ioix{p}")) for p in range(8)]
        init = ec(nc.semaphore("init"))
        gsems = [ec(nc.semaphore(f"gat{j}")) for j in range(cfg.nb_m)]
        sv = ec(nc.semaphore("sv"))
        pe = ec(nc.semaphore("pe"))
        ev = ec(nc.semaphore("ev"))
        osems = [ec(nc.semaphore(f"odma{j}")) for j in range(2)]

        with nc.Block() as block:

            @block.sync
            def _(sync):
                sync.dma_start(slots_sb[:], slots_d[:]).then_inc(iosl, 16)
                qc = idx_cols // 8
                for piece in range(8):
                    lo = piece * qc
                    hi = idx_cols if piece == 7 else (piece + 1) * qc
                    sync.dma_start(
                        idx_sb[:, lo:hi], idx_d[:, lo:hi]
                    ).then_inc(ioix[piece], 16)
                for b in range(nblk):
                    sync.wait_ge(ev, b + 1)
                    sync.dma_start(
                        out_d[:, b * cfg.blk:(b + 1) * cfg.blk], obufs[b % 2][:]
                    ).then_inc(osems[b % 2], 16)
                sync.wait_ge(osems[0], 16 * cdiv(nblk, 2))
                if nblk > 1:
                    sync.wait_ge(osems[1], 16 * (nblk // 2))

            @block.gpsimd
            def _(g):
                g.iota(
                    iota_sb[:], [[1, cfg.sfd]], channel_multiplier=0,
                    allow_small_or_imprecise_dtypes=True,
                ).then_inc(init, 1)
                g.load_library(mlp)
                g.wait_ge(init, 2)
                qc = idx_cols // 8
                pc_seen = -1
                for k in range(ncalls):
                    w = k % W
                    end_col = colpre[k + 1]
                    piece = 7 if end_col > 7 * qc else (end_col - 1) // qc
                    while piece > pc_seen:
                        pc_seen += 1
                        g.wait_ge(ioix[pc_seen], 16)
                    if k >= cfg.nb_m:
                        g.wait_ge(pe, prefix[k - cfg.nb_m + 1] + nblk_before(k - cfg.nb_m + 1, W))
                    rows = min(cfg.win, cfg.n_nodes - w * cfg.win)
                    j = k % cfg.nb_m
                    src_v = nf[w * cfg.win: w * cfg.win + rows, :]
                    g.dma_gather(
                        m_sbs[j][:, 0:cfg.kk[k], :],
                        src_v,
                        idx_sb[:, colpre[k]:colpre[k + 1]],
                        cfg.kk[k] * P,
                        cfg.cmax[k],
                        D,
                        queue_num=k % 4,
                    ).then_inc(gsems[j], 16)

            @block.vector
            def _(v):
                v.wait_ge(init, 1)
                v.memset(szero[:], 0)
                for j in range(cfg.nb_m - 1):
                    v.memset(m_sbs[j][:], 0)
                v.memset(m_sbs[cfg.nb_m - 1][:], 0).then_inc(init, 1)
                v.wait_ge(iosl, 16)
                for k in range(ncalls):
                    if k >= cfg.nb_s:
                        v.wait_ge(pe, prefix[k - cfg.nb_s + 1] + nblk_before(k - cfg.nb_s + 1, W))
                    K = cfg.kk[k]
                    jb = k % cfg.nb_s
                    o = s_sbs[jb][:, 0:K, :]
                    a = iota_sb[:]
                    in0 = AP(a.tensor, a.offset, [list(a.ap[0]), [0, K], list(a.ap[1])])
                    b = slots_sb[:, prefix[k]:prefix[k] + K]
                    in1 = AP(b.tensor, b.offset, [list(b.ap[0]), list(b.ap[1]), [0, cfg.sfd]])
                    v.tensor_tensor(
                        o, in0, in1, mybir.AluOpType.is_equal
                    ).then_inc(sv, 1)

            @block.scalar
            def _(a):
                for b in range(nblk):
                    a.wait_ge(pe, prefix[(b + 1) * W] + (b + 1))
                    if b >= 2:
                        a.wait_ge(osems[b % 2], 16 * (b // 2))
                    a.activation(
                        obufs[b % 2][:], psums[b % 2][:], AF.Copy,
                    ).then_inc(ev, 1)

            @block.tensor
            def _(te):
                for b in range(nblk):
                    for w in range(W):
                        k = b * W + w
                        j = k % cfg.nb_m
                        te.wait_ge(gsems[j], 16 * (k // cfg.nb_m + 1))
                        te.wait_ge(sv, k + 1)
                        if w == 0:
                            if b >= 2:
                                te.wait_ge(ev, b - 1)
                            # zeroing matmul opens the block's accum group
                            te.matmul(
                                psums[b % 2][:],
                                m_sbs[j][:, 0, :],
                                szero[:],
                                start=True,
                                stop=False,
                            ).then_inc(pe, 1)
                        last_call = (w == W - 1)
                        for i in range(cfg.kk[k]):
                            o = cfg.offs[k][i]
                            te.matmul(
                                psums[b % 2][:, o:o + SW],
                                m_sbs[j][:, i, :],
                                s_sbs[k % cfg.nb_s][:, i, 0:SW],
                                start=False,
                                stop=last_call and i == cfg.kk[k] - 1,
                            ).then_inc(pe, 1)

    nc.compile()
    return nc


def nblk_before(ncall, W):
    """Number of block-zeroing matmuls issued before call `ncall` starts."""
    return (ncall + W - 1) // W


def unshard(cfg: Cfg, results, in_norm):
    out = np.empty((cfg.n_nodes, cfg.d), np.float32)
    for c in range(cfg.n_cores):
        o = results[c]["out"]
        out[c * cfg.dpc:(c + 1) * cfg.dpc, :] = o[:, :cfg.dpc].T
    out *= in_norm[:, None]
    return out


def run(inputs, trace=False, **spmd_kwargs):
    from concourse.bass_utils import run_bass_kernel_spmd

    cfg, in_maps, in_norm = prep_host(
        inputs["u_f"], inputs["v_f"], inputs["src"], inputs["dst"]
    )
    nc = build_nc(cfg)
    res = run_bass_kernel_spmd(
        nc, in_maps, core_ids=list(range(cfg.n_cores)), trace=trace,
        **spmd_kwargs,
    )
    return unshard(cfg, res.results, in_norm), res


def kernel(**inputs):
    return run(inputs)[0]
